# revision 1
# baseline (speedup 1.0000x reference)
"""Trainium2 Bass kernel for the sum-product "knowledge layer" network.

Computation (see problem reference):
  h0 = encode(x): 8194-row table [-inf, 0, pos0, neg0, pos1, neg1, ...]
       with pos = x (log-probs), neg = log(1 - exp(x)), per batch column.
  4 alternating layers, each: gather rows by ptrs, then segment-reduce over
  contiguous fanin groups (fanin 4 sum-of-logs "product" layers, fanin 2
  logsumexp "sum" layers).

Strategy (pure batch data-parallelism, 8 NeuronCores):
  - Shard the 512 batch columns 8 ways -> 64 columns per core.
  - Per core every tensor lives in DRAM as [rows, 64] fp32; one row = 256B.
  - Gathers use the SWDGE dma_gather instruction: int16 index list in SBUF,
    each index pulls one 256B row from the DRAM table; index list position j
    lands at SBUF partition j%128, free slot j//128.
  - Host pre-permutes each layer's ptrs so that the edges of output group g
    land on partition g//C (C = n_out/128) at free slots fanin*(g%C)+k.
    Segment reduction then becomes strided free-dim vector ops, and the
    layer output [128, C, 64] DMAs back to DRAM in natural row order
    (partition p holds rows p*C .. p*C+C-1, fully contiguous per partition).
  - Sum layers: logsumexp(a,b) = max + softplus(min - max) on DVE + ACT.
"""

import numpy as np

P = 128
B = 64  # batch columns per core
NCORES = 8
N_VARS = 4096
BATCH = 512
TAB0 = 2 * N_VARS + 2  # 8194
OUT_SIZES = [16384, 8192, 4096, 2048]
FANINS = [4, 2, 4, 2]
CHUNK = 8192  # gather indices per dma_gather instruction


def layer_specs(out_sizes, fanins, tab0):
    specs = []
    prev = tab0
    for n_out, f in zip(out_sizes, fanins):
        specs.append({"f": f, "n_in": prev, "n_out": n_out, "n_edges": n_out * f})
        prev = n_out
    return specs


def reorder_wrap(ptrs, f, n_out):
    """Permute edge pointers into dma_gather order and wrap into the int16
    [128, n_edges//16] SBUF layout (position j -> [j%16, j//16], replicated
    across the 8 gpsimd cores' 16-partition groups)."""
    C = n_out // P
    n_edges = n_out * f
    j = np.arange(n_edges)
    p = j % P
    slot = j // P
    c = slot // f
    k = slot % f
    g = p * C + c
    src = np.asarray(ptrs).astype(np.int64)[g * f + k]
    assert src.max() < 2**15 and src.min() >= 0
    src = src.astype(np.int16)
    return np.ascontiguousarray(np.tile(src.reshape(-1, 16).T, (8, 1)))


def build_nc(n_vars=N_VARS, out_sizes=OUT_SIZES, fanins=FANINS, chunk=CHUNK):
    import concourse.bacc as bacc
    import concourse.mybir as mybir
    import concourse.tile as tile

    f32 = mybir.dt.float32
    i16 = mybir.dt.int16
    Alu = mybir.AluOpType
    Act = mybir.ActivationFunctionType

    tab0 = 2 * n_vars + 2
    specs = layer_specs(out_sizes, fanins, tab0)
    S_ENC = n_vars // P  # encode slots per partition

    nc = bacc.Bacc("TRN2", target_bir_lowering=False, debug=False)
    x = nc.dram_tensor("x", [P, S_ENC * B], f32, kind="ExternalInput")
    idx_in = [
        nc.dram_tensor(f"idx{l}", [P, s["n_edges"] // 16], i16, kind="ExternalInput")
        for l, s in enumerate(specs)
    ]
    out = nc.dram_tensor("out", [out_sizes[-1], B], f32, kind="ExternalOutput")

    with tile.TileContext(nc) as tc:
        with (
            tc.tile_pool(name="dram", bufs=1, space="DRAM") as dpool,
            tc.tile_pool(name="sb", bufs=4) as gp,
            tc.tile_pool(name="hb", bufs=3) as hp,
            tc.tile_pool(name="tmp", bufs=2) as tp,
            tc.tile_pool(name="ix", bufs=1) as ixp,
        ):
            tables = [
                dpool.tile([s["n_in"], B], f32, name=f"t{l}", tag=f"t{l}")
                for l, s in enumerate(specs)
            ]

            # --- index list loads ---
            ix_t = []
            for l, s in enumerate(specs):
                t = ixp.tile([P, s["n_edges"] // 16], i16, tag=f"ix{l}")
                nc.sync.dma_start(t[:], idx_in[l][:])
                ix_t.append(t)

            # --- encode: pos rows at 2+2i, neg rows at 3+2i, zeros at row 1.
            # Partition p computes vars p*S_ENC .. p*S_ENC+S_ENC-1 so the
            # interleaved pos/neg store is one contiguous run per partition.
            iv = gp.tile([P, S_ENC, 2, B], f32, tag="g")
            nc.sync.dma_start(
                iv[:][:, :, 0, :], x[:].rearrange("p (s b) -> p s b", b=B)
            )
            et = hp.tile([P, S_ENC, B], f32, tag="h")
            nc.scalar.activation(et[:], iv[:][:, :, 0, :], Act.Exp)
            nc.scalar.activation(iv[:][:, :, 1, :], et[:], Act.Ln, scale=-1.0, bias=1.0)
            nc.sync.dma_start(
                tables[0][:][2:, :].rearrange("(p s k) b -> p (s k b)", p=P, k=2),
                iv[:].rearrange("p s k b -> p (s k b)"),
            )
            # rows 0 (-inf in the reference, never gathered) and 1 (zeros)
            z = ixp.tile([2, B], f32, tag="z")
            nc.vector.memset(z[:], 0.0)
            nc.sync.dma_start(tables[0][:][0:2, :], z[:])

            # --- gather + segment-reduce layers ---
            for l, s in enumerate(specs):
                f, n_out, n_edges = s["f"], s["n_out"], s["n_edges"]
                C = n_out // P
                ch = min(chunk if f == 4 else chunk // 2, n_edges)
                assert n_edges % ch == 0
                S = ch // P  # slots per chunk
                Csub = S // f  # groups per partition per chunk
                src_ap = tables[l][:]
                dst_full = (tables[l + 1][:] if l + 1 < len(specs) else out[:]).rearrange(
                    "(p C) b -> p C b", p=P
                )
                for ci in range(n_edges // ch):
                    g = gp.tile([P, S, B], f32, tag="g")
                    nc.gpsimd.dma_gather(
                        g[:],
                        src_ap,
                        ix_t[l][:, ci * (ch // 16) : (ci + 1) * (ch // 16)],
                        ch,
                        ch,
                        B,
                        single_packet=False,
                    )
                    v = g[:].rearrange("p (c k) b -> p c k b", k=f)
                    h = hp.tile([P, Csub, B], f32, tag="h")
                    if f == 4:
                        s01 = tp.tile([P, Csub, B], f32, tag="m")
                        s23 = tp.tile([P, Csub, B], f32, tag="n")
                        nc.vector.tensor_add(s01[:], v[:, :, 0, :], v[:, :, 1, :])
                        nc.vector.tensor_add(s23[:], v[:, :, 2, :], v[:, :, 3, :])
                        nc.vector.tensor_add(h[:], s01[:], s23[:])
                    else:
                        # logsumexp(a,b) = max + ln(1 + exp(min - max))
                        m = tp.tile([P, Csub, B], f32, tag="m")
                        mn = tp.tile([P, Csub, B], f32, tag="n")
                        d = tp.tile([P, Csub, B], f32, tag="d")
                        sp = tp.tile([P, Csub, B], f32, tag="sp")
                        nc.vector.tensor_tensor(
                            m[:], v[:, :, 0, :], v[:, :, 1, :], op=Alu.max
                        )
                        nc.vector.tensor_tensor(
                            mn[:], v[:, :, 0, :], v[:, :, 1, :], op=Alu.min
                        )
                        nc.vector.tensor_tensor(d[:], mn[:], m[:], op=Alu.subtract)
                        nc.scalar.activation(d[:], d[:], Act.Exp)
                        nc.scalar.activation(sp[:], d[:], Act.Ln, bias=1.0)
                        nc.vector.tensor_add(h[:], m[:], sp[:])
                    nc.sync.dma_start(
                        dst_full[:, ci * Csub : (ci + 1) * Csub, :], h[:]
                    )
    nc.compile()
    return nc


def host_prep(x, ptrs_list, seg_list, n_vars=N_VARS, out_sizes=OUT_SIZES, fanins=FANINS):
    """Host-side sharding + index preprocessing. Returns per-core input maps."""
    x = np.asarray(x, dtype=np.float32)
    specs = layer_specs(out_sizes, fanins, 2 * n_vars + 2)
    idx_maps = {}
    for l, s in enumerate(specs):
        seg = np.asarray(seg_list[l]).astype(np.int64)
        expected = np.repeat(np.arange(s["n_out"], dtype=np.int64), s["f"])
        assert np.array_equal(seg, expected), f"layer {l}: non-uniform segments"
        idx_maps[f"idx{l}"] = reorder_wrap(ptrs_list[l], s["f"], s["n_out"])

    batch = x.shape[1]
    bpc = batch // NCORES
    in_maps = []
    for i in range(NCORES):
        xs = x[:, i * bpc : (i + 1) * bpc]
        # partition p holds vars p*S_ENC .. p*S_ENC+S_ENC-1 (natural order)
        xv = np.ascontiguousarray(xs).reshape(P, -1)
        in_maps.append({"x": xv, **idx_maps})
    return in_maps


_CACHE = {}


def _get_nc():
    if "nc" not in _CACHE:
        _CACHE["nc"] = build_nc()
    return _CACHE["nc"]


def kernel(x, ptrs0, seg0, ptrs1, seg1, ptrs2, seg2, ptrs3, seg3):
    from concourse.bass_utils import run_bass_kernel_spmd

    nc = _get_nc()
    in_maps = host_prep(
        x, [ptrs0, ptrs1, ptrs2, ptrs3], [seg0, seg1, seg2, seg3]
    )
    res = run_bass_kernel_spmd(nc, in_maps, core_ids=list(range(NCORES)))
    outs = [r["out"] for r in res.results]
    return np.concatenate(outs, axis=1)



# revision 2
# speedup vs baseline: 1.7270x; 1.7270x over previous
"""Trainium2 Bass kernel for the sum-product "knowledge layer" network.

Computation (see problem reference):
  h0 = encode(x): 8194-row table [-inf, 0, pos0, neg0, ...] with pos = x
       (log-probs), neg = log(1 - exp(x)), per batch column.
  4 alternating layers: gather rows by ptrs, segment-reduce over contiguous
  fanin groups (fanin-4 sum "product" layers, fanin-2 logsumexp "sum" layers).

Strategy (pure batch data-parallelism, 8 NeuronCores, SBUF-resident):
  - Shard the 512 batch columns 8 ways -> 64 columns per core.
  - Batch-on-partition layout: every table lives in SBUF as [128, n_rows]
    f32 where channel c holds batch column c%64 (two identical copies).
  - Gathers use the gpsimd ap_gather instruction (SBUF->SBUF, per-16-
    partition-group index lists): channels 0-63 gather edge-set A while
    64-127 gather edge-set B, so each free slot delivers 2 edges.
  - Layer fusion: t1 rows are consumed on average once by layer 1 and t3
    rows once by layer 3, so layers (0,1) and (2,3) are fused.  The host
    composes the pointer chains: one device gather phase fetches, per fused
    output group g, the 8 t0-rows {ptr0[4*ptr1[2g+j]+k]}, the device then
    computes lse(sum4, sum4).  Phase 2 repeats the pattern from t2.
  - Encode avoids interleaving: t0 = [2 const rows | pos block | neg block]
    and the host remaps ptr0 accordingly; x DMA-loads straight into the pos
    block, Act writes exp/log1mexp into the neg block.
"""

import numpy as np

P = 128
B = 64  # batch columns per core
NCORES = 8
N_VARS = 4096
BATCH = 512
TAB0 = 2 * N_VARS + 2  # 8194
OUT_SIZES = [16384, 8192, 4096, 2048]
FANINS = [4, 2, 4, 2]
N_MID = 8192  # t2 rows (output of fused phase 1)
N_OUT = 2048
P1_CHUNKS = 4
P1_GROUPS = N_MID // P1_CHUNKS  # fused groups per phase-1 chunk (2048)
SLOTS = 8192  # gather slots per ap_gather instruction


def wrap128(list_a, list_b):
    """Pack two per-half edge lists (len S each) into the ap_gather int16
    index layout [128, S//16]: position j of a group's list sits at
    [16*grp + j%16, j//16]; groups 0-3 share list A, 4-7 share list B."""
    a = np.asarray(list_a, np.int64)
    b = np.asarray(list_b, np.int64)
    assert a.size == b.size and a.size % 16 == 0
    wa = np.tile(a.reshape(-1, 16).T, (4, 1))
    wb = np.tile(b.reshape(-1, 16).T, (4, 1))
    w = np.concatenate([wa, wb], axis=0)
    assert w.min() >= 0 and w.max() < 2**15
    return np.ascontiguousarray(w.astype(np.int16))


def build_nc():
    import concourse.bacc as bacc
    import concourse.mybir as mybir
    import concourse.tile as tile

    f32 = mybir.dt.float32
    i16 = mybir.dt.int16
    Alu = mybir.AluOpType
    Act = mybir.ActivationFunctionType

    nc = bacc.Bacc("TRN2", target_bir_lowering=False, debug=False)
    xv_in = nc.dram_tensor("xv", [P, N_VARS], f32, kind="ExternalInput")
    idx1_in = [
        nc.dram_tensor(f"idx1_{c}", [P, SLOTS // 16], i16, kind="ExternalInput")
        for c in range(P1_CHUNKS)
    ]
    idx2_in = nc.dram_tensor("idx2", [P, SLOTS // 16], i16, kind="ExternalInput")
    out_d = nc.dram_tensor("out", [B, N_OUT], f32, kind="ExternalOutput")

    with tile.TileContext(nc) as tc:
        with (
            tc.tile_pool(name="tabs", bufs=1) as tabs,
            tc.tile_pool(name="gp", bufs=2) as gp,
            tc.tile_pool(name="tp", bufs=1) as tp,
            tc.tile_pool(name="ix", bufs=1) as ixp,
        ):
            t0 = tabs.tile([P, TAB0], f32, tag="t0")
            t2 = tabs.tile([P, N_MID], f32, tag="t2")
            ot = tabs.tile([P, N_OUT], f32, tag="ot")
            ix1 = [
                ixp.tile([P, SLOTS // 16], i16, tag=f"i{c}", name=f"ix1_{c}")
                for c in range(P1_CHUNKS)
            ]
            ix2 = ixp.tile([P, SLOTS // 16], i16, tag="i4")
            for c in range(P1_CHUNKS):
                nc.sync.dma_start(ix1[c][:], idx1_in[c][:])
            nc.sync.dma_start(ix2[:], idx2_in[:])

            # --- encode: t0 = [0, 0 | pos_0..pos_4095 | neg_0..neg_4095] ---
            nc.vector.memset(t0[:][:, 0:2], 0.0)
            nc.sync.dma_start(t0[:][:, 2 : 2 + N_VARS], xv_in[:])
            half = N_VARS // 2
            for h in range(2):
                et = tp.tile([P, half], f32, tag="et", name=f"et{h}")
                pos = t0[:][:, 2 + half * h : 2 + half * (h + 1)]
                neg = t0[:][:, 2 + N_VARS + half * h : 2 + N_VARS + half * (h + 1)]
                nc.scalar.activation(et[:], pos, Act.Exp)
                nc.scalar.activation(neg, et[:], Act.Ln, scale=-1.0, bias=1.0)

            def reduce8_lse(g, dst, row0, rows_half, subs=2):
                """g [128, 8*rows_half]: per channel-half, rows_half fused
                groups of 8 slots [a0..a3, b0..b3]; writes
                lse(a0+..+a3, b0+..+b3) to dst rows [row0, row0+2*rows_half)
                (half A on channels 0-63, half B on 64-127)."""
                sg = rows_half // subs  # groups per sub-chunk
                for s in range(subs):
                    gs = (
                        g[:][:, 8 * sg * s : 8 * sg * (s + 1)]
                        .rearrange("p (h k) -> p h k", k=8)
                    )
                    u = tp.tile([P, sg, 4], f32, tag="u", name=f"u{row0}_{s}")
                    nc.vector.tensor_add(u[:], gs[:, :, 0::2], gs[:, :, 1::2])
                    w = tp.tile([P, sg, 2], f32, tag="w", name=f"w{row0}_{s}")
                    nc.vector.tensor_add(w[:], u[:][:, :, 0::2], u[:][:, :, 1::2])
                    m = tp.tile([P, sg], f32, tag="m", name=f"m{row0}_{s}")
                    mn = tp.tile([P, sg], f32, tag="n", name=f"n{row0}_{s}")
                    nc.vector.tensor_tensor(
                        m[:], w[:][:, :, 0], w[:][:, :, 1], op=Alu.max
                    )
                    nc.vector.tensor_tensor(
                        mn[:], w[:][:, :, 0], w[:][:, :, 1], op=Alu.min
                    )
                    nc.vector.tensor_tensor(mn[:], mn[:], m[:], op=Alu.subtract)
                    nc.scalar.activation(mn[:], mn[:], Act.Exp)
                    nc.scalar.activation(mn[:], mn[:], Act.Ln, bias=1.0)
                    ra = slice(row0 + sg * s, row0 + sg * (s + 1))
                    rb = slice(row0 + rows_half + sg * s, row0 + rows_half + sg * (s + 1))
                    nc.vector.tensor_add(dst[:][0:B, ra], m[:][0:B, :], mn[:][0:B, :])
                    nc.vector.tensor_add(
                        dst[:][B:P, rb], m[:][B:P, :], mn[:][B:P, :]
                    )

            # --- phase 1: fused layers 0+1 -> t2 ---
            hg = P1_GROUPS // 2  # groups per half per chunk (1024)
            for ci in range(P1_CHUNKS):
                g = gp.tile([P, SLOTS], f32, tag="g", name=f"g1_{ci}")
                nc.gpsimd.ap_gather(
                    g[:],
                    t0[:].rearrange("p (n d) -> p n d", d=1),
                    ix1[ci][:],
                    P,
                    TAB0,
                    1,
                    SLOTS,
                )
                row0 = P1_GROUPS * ci
                reduce8_lse(g, t2, row0, hg)
                # replicate each half's rows into the other channel copy
                nc.sync.dma_start(
                    t2[:][B:P, row0 : row0 + hg], t2[:][0:B, row0 : row0 + hg]
                )
                nc.sync.dma_start(
                    t2[:][0:B, row0 + hg : row0 + 2 * hg],
                    t2[:][B:P, row0 + hg : row0 + 2 * hg],
                )

            # --- phase 2: fused layers 2+3 -> out ---
            g = gp.tile([P, SLOTS], f32, tag="g", name="g2")
            nc.gpsimd.ap_gather(
                g[:],
                t2[:].rearrange("p (n d) -> p n d", d=1),
                ix2[:],
                P,
                N_MID,
                1,
                SLOTS,
            )
            reduce8_lse(g, ot, 0, N_OUT // 2)
            nc.sync.dma_start(out_d[:][:, 0 : N_OUT // 2], ot[:][0:B, 0 : N_OUT // 2])
            nc.sync.dma_start(
                out_d[:][:, N_OUT // 2 : N_OUT], ot[:][B:P, N_OUT // 2 : N_OUT]
            )
    nc.compile()
    return nc


def host_prep(x, ptrs_list, seg_list, n_vars=N_VARS):
    """Host-side sharding + pointer-chain composition. Returns per-core
    input maps."""
    x = np.asarray(x, dtype=np.float32)
    p0, p1, p2, p3 = [np.asarray(p).astype(np.int64) for p in ptrs_list]
    for i, (n_out, f) in enumerate(zip(OUT_SIZES, FANINS)):
        seg = np.asarray(seg_list[i]).astype(np.int64)
        expected = np.repeat(np.arange(n_out, dtype=np.int64), f)
        assert np.array_equal(seg, expected), f"layer {i}: non-uniform segments"

    # remap ptr0 rows into the block layout [0, 0 | pos | neg]
    q0 = np.where(
        p0 < 2, p0, np.where(p0 % 2 == 0, 2 + (p0 - 2) // 2, 2 + n_vars + (p0 - 3) // 2)
    )

    k4 = np.arange(4)
    # phase 1: fused group g (t2 row g): 8 t0-rows q0[4*p1[2g]+k], q0[4*p1[2g+1]+k]
    a, b = p1[0::2], p1[1::2]
    g1 = np.concatenate(
        [q0[4 * a[:, None] + k4], q0[4 * b[:, None] + k4]], axis=1
    )  # [8192, 8]
    # phase 2: fused group h (out row h): 8 t2-rows p2[4*p3[2h]+k], p2[4*p3[2h+1]+k]
    c, d = p3[0::2], p3[1::2]
    g2 = np.concatenate(
        [p2[4 * c[:, None] + k4], p2[4 * d[:, None] + k4]], axis=1
    )  # [2048, 8]

    idx_maps = {}
    for ci in range(P1_CHUNKS):
        gr = g1[P1_GROUPS * ci : P1_GROUPS * (ci + 1)]
        hg = P1_GROUPS // 2
        idx_maps[f"idx1_{ci}"] = wrap128(gr[:hg].reshape(-1), gr[hg:].reshape(-1))
    idx_maps["idx2"] = wrap128(
        g2[: N_OUT // 2].reshape(-1), g2[N_OUT // 2 :].reshape(-1)
    )

    batch = x.shape[1]
    bpc = batch // NCORES
    in_maps = []
    for i in range(NCORES):
        xt = np.ascontiguousarray(x[:, i * bpc : (i + 1) * bpc].T)  # [64, 4096]
        xv = np.concatenate([xt, xt], axis=0)  # [128, 4096], both copies
        in_maps.append({"xv": xv, **idx_maps})
    return in_maps


_CACHE = {}


def _get_nc():
    if "nc" not in _CACHE:
        _CACHE["nc"] = build_nc()
    return _CACHE["nc"]


def kernel(x, ptrs0, seg0, ptrs1, seg1, ptrs2, seg2, ptrs3, seg3):
    from concourse.bass_utils import run_bass_kernel_spmd

    nc = _get_nc()
    in_maps = host_prep(
        x, [ptrs0, ptrs1, ptrs2, ptrs3], [seg0, seg1, seg2, seg3]
    )
    res = run_bass_kernel_spmd(nc, in_maps, core_ids=list(range(NCORES)))
    outs = [r["out"] for r in res.results]  # each [64, 2048]
    return np.concatenate([o.T for o in outs], axis=1).astype(np.float32)


# revision 7
# speedup vs baseline: 2.0653x; 1.1959x over previous
"""Trainium2 Bass kernel for the sum-product "knowledge layer" network.

Computation (see problem reference):
  h0 = encode(x): 8194-row table [-inf, 0, pos0, neg0, ...] with pos = x
       (log-probs), neg = log(1 - exp(x)), per batch column.
  4 alternating layers: gather rows by ptrs, segment-reduce over contiguous
  fanin groups (fanin-4 sum "product" layers, fanin-2 logsumexp "sum" layers).

Strategy (pure batch data-parallelism, 8 NeuronCores, SBUF-resident):
  - Shard the 512 batch columns 8 ways -> 64 columns per core.
  - Batch-on-partition layout: every table lives in SBUF as [128, n_rows]
    f32 where channel c holds batch column c%64 (two identical copies).
  - Gathers use the gpsimd ap_gather instruction (SBUF->SBUF, per-16-
    partition-group index lists): channels 0-63 gather edge-set A while
    64-127 gather edge-set B, so each free slot delivers 2 edges.
  - Layer fusion: t1 rows are consumed on average once by layer 1 and t3
    rows once by layer 3, so layers (0,1) and (2,3) are fused.  The host
    composes the pointer chains: one device gather phase fetches, per fused
    output group g, the 8 t0-rows {ptr0[4*ptr1[2g+j]+k]}, the device then
    computes lse(sum4, sum4).  Phase 2 repeats the pattern from t2.
  - Encode avoids interleaving: t0 = [2 const rows | pos block | neg block]
    and the host remaps ptr0 accordingly; x DMA-loads straight into the pos
    block, Act writes exp/log1mexp into the neg block.
"""

import numpy as np

P = 128
B = 64  # batch columns per core
NCORES = 8
N_VARS = 4096
BATCH = 512
TAB0 = 2 * N_VARS + 2  # 8194
OUT_SIZES = [16384, 8192, 4096, 2048]
FANINS = [4, 2, 4, 2]
N_MID = 8192  # t2 rows (output of fused phase 1)
N_OUT = 2048
P1_CHUNKS = 4
P1_GROUPS = N_MID // P1_CHUNKS  # fused groups per phase-1 chunk (2048)
SLOTS = 8192  # gather slots per ap_gather instruction


def wrap128(list_a, list_b):
    """Pack two per-half edge lists (len S each) into the ap_gather int16
    index layout [128, S//16]: position j of a group's list sits at
    [16*grp + j%16, j//16]; groups 0-3 share list A, 4-7 share list B."""
    a = np.asarray(list_a, np.int64)
    b = np.asarray(list_b, np.int64)
    assert a.size == b.size and a.size % 16 == 0
    wa = np.tile(a.reshape(-1, 16).T, (4, 1))
    wb = np.tile(b.reshape(-1, 16).T, (4, 1))
    w = np.concatenate([wa, wb], axis=0)
    assert w.min() >= 0 and w.max() < 2**15
    return np.ascontiguousarray(w.astype(np.int16))


def _patch_act_tables():
    """Make the combined exp+ln activation-function set the only candidate
    for Exp and Ln so the compiler emits a single LoadActFuncSet instead of
    ping-ponging between the exp-only and ln-only sets (1283ns per reload).
    Set ids (dict order) are preserved; the combined set genuinely contains
    both functions, so device behavior is unchanged."""
    import concourse.bacc as bacc
    import concourse.hw_specs as hws
    import concourse.mybir as mybir

    Act = mybir.ActivationFunctionType
    orig = hws.get_activation_tables

    def patched(arch):
        tabs = dict(orig(arch))
        out = {}
        for name, s in tabs.items():
            s2 = set(s)
            if name != "natural_log_exp_and_others":
                s2.discard(Act.Exp)
                s2.discard(Act.Ln)
            out[name] = s2
        return out

    bacc.get_activation_tables = patched


def build_nc():
    import concourse.bacc as bacc
    import concourse.mybir as mybir
    import concourse.tile as tile

    _patch_act_tables()

    f32 = mybir.dt.float32
    i16 = mybir.dt.int16
    Alu = mybir.AluOpType
    Act = mybir.ActivationFunctionType

    nc = bacc.Bacc("TRN2", target_bir_lowering=False, debug=False)
    xv_in = nc.dram_tensor("xv", [P, N_VARS], f32, kind="ExternalInput")
    idx1_in = [
        nc.dram_tensor(f"idx1_{c}", [P, SLOTS // 16], i16, kind="ExternalInput")
        for c in range(P1_CHUNKS)
    ]
    idx2_in = nc.dram_tensor("idx2", [P, SLOTS // 16], i16, kind="ExternalInput")
    out_d = nc.dram_tensor("out", [B, N_OUT], f32, kind="ExternalOutput")

    with tile.TileContext(nc) as tc:
        with (
            tc.tile_pool(name="tabs", bufs=1) as tabs,
            tc.tile_pool(name="gp", bufs=2) as gp,
            tc.tile_pool(name="tp", bufs=2) as tp,
            tc.tile_pool(name="ix", bufs=1) as ixp,
        ):
            t0 = tabs.tile([P, TAB0], f32, tag="t0")
            t2 = tabs.tile([P, N_MID], f32, tag="t2")
            ot = tabs.tile([P, N_OUT], f32, tag="ot")
            ix1 = [
                ixp.tile([P, SLOTS // 16], i16, tag=f"i{c}", name=f"ix1_{c}")
                for c in range(P1_CHUNKS)
            ]
            ix2 = ixp.tile([P, SLOTS // 16], i16, tag="i4")
            for c in range(P1_CHUNKS):
                nc.sync.dma_start(ix1[c][:], idx1_in[c][:])
            nc.sync.dma_start(ix2[:], idx2_in[:])

            # --- encode: t0 = [0, 0 | pos_0..pos_4095 | neg_0..neg_4095] ---
            nc.vector.memset(t0[:][:, 0:2], 0.0)
            nc.sync.dma_start(t0[:][:, 2 : 2 + N_VARS], xv_in[:])
            half = N_VARS // 2
            for h in range(2):
                et = tp.tile([P, half], f32, tag="et", name=f"et{h}")
                pos = t0[:][:, 2 + half * h : 2 + half * (h + 1)]
                neg = t0[:][:, 2 + N_VARS + half * h : 2 + N_VARS + half * (h + 1)]
                nc.scalar.activation(et[:], pos, Act.Exp)
                nc.scalar.activation(neg, et[:], Act.Ln, scale=-1.0, bias=1.0)

            def reduce8_lse(g, dst, row0, rows_half, replicate, subs=2):
                """g [128, 8*rows_half]: per channel-half, rows_half fused
                groups of 8 slots [a0..a3, b0..b3]; writes
                lse(a0+..+a3, b0+..+b3) to dst rows [row0, row0+2*rows_half)
                (half A from channels 0-63, half B from 64-127).  With
                replicate=True both channel copies of dst get every row (DMA
                placement copies); otherwise each half lands only on its own
                channels."""
                sg = rows_half // subs  # groups per sub-chunk
                for s in range(subs):
                    gs = (
                        g[:][:, 8 * sg * s : 8 * sg * (s + 1)]
                        .rearrange("p (h k) -> p h k", k=8)
                    )
                    u = tp.tile([P, sg, 4], f32, tag="u", name=f"u{row0}_{s}")
                    nc.vector.tensor_add(u[:], gs[:, :, 0::2], gs[:, :, 1::2])
                    w = tp.tile([P, sg, 2], f32, tag="w", name=f"w{row0}_{s}")
                    nc.vector.tensor_add(w[:], u[:][:, :, 0::2], u[:][:, :, 1::2])
                    m = tp.tile([P, sg], f32, tag="m", name=f"m{row0}_{s}")
                    mn = tp.tile([P, sg], f32, tag="n", name=f"n{row0}_{s}")
                    nc.vector.tensor_tensor(
                        m[:], w[:][:, :, 0], w[:][:, :, 1], op=Alu.max
                    )
                    nc.vector.tensor_tensor(
                        mn[:], w[:][:, :, 0], w[:][:, :, 1], op=Alu.min
                    )
                    nc.vector.tensor_tensor(mn[:], mn[:], m[:], op=Alu.subtract)
                    nc.scalar.activation(mn[:], mn[:], Act.Exp)
                    nc.scalar.activation(mn[:], mn[:], Act.Ln, bias=1.0)
                    h = tp.tile([P, sg], f32, tag="h", name=f"h{row0}_{s}")
                    nc.vector.tensor_add(h[:], m[:], mn[:])
                    ra = slice(row0 + sg * s, row0 + sg * (s + 1))
                    rb = slice(row0 + rows_half + sg * s, row0 + rows_half + sg * (s + 1))
                    nc.sync.dma_start(dst[:][0:B, ra], h[:][0:B, :])
                    nc.sync.dma_start(dst[:][B:P, rb], h[:][B:P, :])
                    if replicate:
                        nc.sync.dma_start(dst[:][B:P, ra], h[:][0:B, :])
                        nc.sync.dma_start(dst[:][0:B, rb], h[:][B:P, :])

            # --- phase 1: fused layers 0+1 -> t2 ---
            hg = P1_GROUPS // 2  # groups per half per chunk (1024)
            for ci in range(P1_CHUNKS):
                g = gp.tile([P, SLOTS], f32, tag="g", name=f"g1_{ci}")
                nc.gpsimd.ap_gather(
                    g[:],
                    t0[:].rearrange("p (n d) -> p n d", d=1),
                    ix1[ci][:],
                    P,
                    TAB0,
                    1,
                    SLOTS,
                )
                row0 = P1_GROUPS * ci
                reduce8_lse(g, t2, row0, hg, replicate=True)

            # --- phase 2: fused layers 2+3 -> out ---
            g = gp.tile([P, SLOTS], f32, tag="g", name="g2")
            nc.gpsimd.ap_gather(
                g[:],
                t2[:].rearrange("p (n d) -> p n d", d=1),
                ix2[:],
                P,
                N_MID,
                1,
                SLOTS,
            )
            reduce8_lse(g, ot, 0, N_OUT // 2, replicate=False)
            nc.sync.dma_start(out_d[:][:, 0 : N_OUT // 2], ot[:][0:B, 0 : N_OUT // 2])
            nc.sync.dma_start(
                out_d[:][:, N_OUT // 2 : N_OUT], ot[:][B:P, N_OUT // 2 : N_OUT]
            )
    nc.compile()
    return nc


def host_prep(x, ptrs_list, seg_list, n_vars=N_VARS):
    """Host-side sharding + pointer-chain composition. Returns per-core
    input maps."""
    x = np.asarray(x, dtype=np.float32)
    p0, p1, p2, p3 = [np.asarray(p).astype(np.int64) for p in ptrs_list]
    for i, (n_out, f) in enumerate(zip(OUT_SIZES, FANINS)):
        seg = np.asarray(seg_list[i]).astype(np.int64)
        expected = np.repeat(np.arange(n_out, dtype=np.int64), f)
        assert np.array_equal(seg, expected), f"layer {i}: non-uniform segments"

    # remap ptr0 rows into the block layout [0, 0 | pos | neg]
    q0 = np.where(
        p0 < 2, p0, np.where(p0 % 2 == 0, 2 + (p0 - 2) // 2, 2 + n_vars + (p0 - 3) // 2)
    )

    k4 = np.arange(4)
    # phase 1: fused group g (t2 row g): 8 t0-rows q0[4*p1[2g]+k], q0[4*p1[2g+1]+k]
    a, b = p1[0::2], p1[1::2]
    g1 = np.concatenate(
        [q0[4 * a[:, None] + k4], q0[4 * b[:, None] + k4]], axis=1
    )  # [8192, 8]
    # phase 2: fused group h (out row h): 8 t2-rows p2[4*p3[2h]+k], p2[4*p3[2h+1]+k]
    c, d = p3[0::2], p3[1::2]
    g2 = np.concatenate(
        [p2[4 * c[:, None] + k4], p2[4 * d[:, None] + k4]], axis=1
    )  # [2048, 8]

    idx_maps = {}
    for ci in range(P1_CHUNKS):
        gr = g1[P1_GROUPS * ci : P1_GROUPS * (ci + 1)]
        hg = P1_GROUPS // 2
        idx_maps[f"idx1_{ci}"] = wrap128(gr[:hg].reshape(-1), gr[hg:].reshape(-1))
    idx_maps["idx2"] = wrap128(
        g2[: N_OUT // 2].reshape(-1), g2[N_OUT // 2 :].reshape(-1)
    )

    batch = x.shape[1]
    bpc = batch // NCORES
    in_maps = []
    for i in range(NCORES):
        xt = np.ascontiguousarray(x[:, i * bpc : (i + 1) * bpc].T)  # [64, 4096]
        xv = np.concatenate([xt, xt], axis=0)  # [128, 4096], both copies
        in_maps.append({"xv": xv, **idx_maps})
    return in_maps


_CACHE = {}


def _get_nc():
    if "nc" not in _CACHE:
        _CACHE["nc"] = build_nc()
    return _CACHE["nc"]


def kernel(x, ptrs0, seg0, ptrs1, seg1, ptrs2, seg2, ptrs3, seg3):
    from concourse.bass_utils import run_bass_kernel_spmd

    nc = _get_nc()
    in_maps = host_prep(
        x, [ptrs0, ptrs1, ptrs2, ptrs3], [seg0, seg1, seg2, seg3]
    )
    res = run_bass_kernel_spmd(nc, in_maps, core_ids=list(range(NCORES)))
    outs = [r["out"] for r in res.results]  # each [64, 2048]
    return np.concatenate([o.T for o in outs], axis=1).astype(np.float32)


# revision 12
# speedup vs baseline: 2.2204x; 1.0751x over previous
"""Trainium2 Bass kernel for the sum-product "knowledge layer" network.

Computation (see problem reference):
  h0 = encode(x): 8194-row table [-inf, 0, pos0, neg0, ...] with pos = x
       (log-probs), neg = log(1 - exp(x)), per batch column.
  4 alternating layers: gather rows by ptrs, segment-reduce over contiguous
  fanin groups (fanin-4 sum "product" layers, fanin-2 logsumexp "sum" layers).

Strategy (pure batch data-parallelism, 8 NeuronCores, SBUF-resident):
  - Shard the 512 batch columns 8 ways -> 64 columns per core.
  - Batch-on-partition layout: every table lives in SBUF as [128, n_rows]
    f32 where channel c holds batch column c%64 (two identical copies).
  - Gathers use the gpsimd ap_gather instruction (SBUF->SBUF, per-16-
    partition-group index lists): channels 0-63 gather edge-set A while
    64-127 gather edge-set B, so each free slot delivers 2 edges.
  - Layer fusion: t1 rows are consumed on average once by layer 1 and t3
    rows once by layer 3, so layers (0,1) and (2,3) are fused.  The host
    composes the pointer chains: one device gather phase fetches, per fused
    output group g, the 8 t0-rows {ptr0[4*ptr1[2g+j]+k]}, the device then
    computes lse(sum4, sum4).  Phase 2 repeats the pattern from t2.
  - Encode avoids interleaving: t0 = [2 const rows | pos block | neg block]
    and the host remaps ptr0 accordingly; x DMA-loads straight into the pos
    block, Act writes exp/log1mexp into the neg block.
"""

import numpy as np

P = 128
B = 64  # batch columns per core
NCORES = 8
N_VARS = 4096
BATCH = 512
TAB0 = 2 * N_VARS + 2  # 8194
OUT_SIZES = [16384, 8192, 4096, 2048]
FANINS = [4, 2, 4, 2]
N_MID = 8192  # t2 rows (output of fused phase 1)
N_OUT = 2048
P1_CHUNKS = 4
P1_GROUPS = N_MID // P1_CHUNKS  # fused groups per phase-1 chunk (2048)
SLOTS = 8192  # gather slots per ap_gather instruction


def wrap128(list_a, list_b):
    """Pack two per-half edge lists (len S each) into the ap_gather int16
    index layout [128, S//16]: position j of a group's list sits at
    [16*grp + j%16, j//16]; groups 0-3 share list A, 4-7 share list B."""
    a = np.asarray(list_a, np.int64)
    b = np.asarray(list_b, np.int64)
    assert a.size == b.size and a.size % 16 == 0
    wa = np.tile(a.reshape(-1, 16).T, (4, 1))
    wb = np.tile(b.reshape(-1, 16).T, (4, 1))
    w = np.concatenate([wa, wb], axis=0)
    assert w.min() >= 0 and w.max() < 2**15
    return np.ascontiguousarray(w.astype(np.int16))


def _patch_act_tables():
    """Make the combined exp+ln activation-function set the only candidate
    for Exp and Ln so the compiler emits a single LoadActFuncSet instead of
    ping-ponging between the exp-only and ln-only sets (1283ns per reload).
    Set ids (dict order) are preserved; the combined set genuinely contains
    both functions, so device behavior is unchanged."""
    import concourse.bacc as bacc
    import concourse.hw_specs as hws
    import concourse.mybir as mybir

    Act = mybir.ActivationFunctionType
    orig = hws.get_activation_tables

    def patched(arch):
        tabs = dict(orig(arch))
        out = {}
        for name, s in tabs.items():
            s2 = set(s)
            if name != "natural_log_exp_and_others":
                s2.discard(Act.Exp)
                s2.discard(Act.Ln)
            out[name] = s2
        return out

    bacc.get_activation_tables = patched


def build_nc():
    import concourse.bacc as bacc
    import concourse.mybir as mybir
    import concourse.tile as tile

    _patch_act_tables()

    f32 = mybir.dt.float32
    i16 = mybir.dt.int16
    Alu = mybir.AluOpType
    Act = mybir.ActivationFunctionType

    nc = bacc.Bacc("TRN2", target_bir_lowering=False, debug=False)
    xv_in = nc.dram_tensor("xv", [P, N_VARS], f32, kind="ExternalInput")
    idx1_in = [
        nc.dram_tensor(f"idx1_{c}", [P, SLOTS // 16], i16, kind="ExternalInput")
        for c in range(P1_CHUNKS)
    ]
    idx2_in = nc.dram_tensor("idx2", [P, SLOTS // 16], i16, kind="ExternalInput")
    out_d = nc.dram_tensor("out", [B, N_OUT], f32, kind="ExternalOutput")

    with tile.TileContext(nc) as tc:
        with (
            tc.tile_pool(name="tabs", bufs=1) as tabs,
            tc.tile_pool(name="gp", bufs=2) as gp,
            tc.tile_pool(name="tp", bufs=2) as tp,
            tc.tile_pool(name="ix", bufs=1) as ixp,
        ):
            t0 = tabs.tile([P, TAB0], f32, tag="t0")
            t2 = tabs.tile([P, N_MID], f32, tag="t2")

            # --- encode: t0 = [0, 0 | pos_0..pos_4095 | neg_0..neg_4095] ---
            # x loads go first (the encode chain is the critical-path head);
            # halves let Exp start while the second half is still in flight.
            half = N_VARS // 2
            nc.vector.memset(t0[:][:, 0:2], 0.0)
            for h in range(2):
                nc.sync.dma_start(
                    t0[:][:, 2 + half * h : 2 + half * (h + 1)],
                    xv_in[:][:, half * h : half * (h + 1)],
                )
            ix1 = [
                ixp.tile([P, SLOTS // 16], i16, tag=f"i{c}", name=f"ix1_{c}")
                for c in range(P1_CHUNKS)
            ]
            ix2 = ixp.tile([P, SLOTS // 16], i16, tag="i4")
            for c in range(P1_CHUNKS):
                nc.sync.dma_start(ix1[c][:], idx1_in[c][:])
            nc.sync.dma_start(ix2[:], idx2_in[:])
            for h in range(2):
                et = tp.tile([P, half], f32, tag="et", name=f"et{h}")
                pos = t0[:][:, 2 + half * h : 2 + half * (h + 1)]
                neg = t0[:][:, 2 + N_VARS + half * h : 2 + N_VARS + half * (h + 1)]
                nc.scalar.activation(et[:], pos, Act.Exp)
                nc.scalar.activation(neg, et[:], Act.Ln, scale=-1.0, bias=1.0)

            def reduce8_lse(g, dst, row0, rows_half, replicate, subs=2):
                """g [128, 8*rows_half]: per channel-half, rows_half fused
                groups of 8 slots [a0..a3, b0..b3]; writes
                lse(a0+..+a3, b0+..+b3) to dst rows [row0, row0+2*rows_half)
                (half A from channels 0-63, half B from 64-127).  dst may be
                an SBUF table tile or a DRAM [64, n] output.  With
                replicate=True both channel copies of dst get every row (DMA
                placement copies); otherwise each half lands only on its own
                channels."""
                sg = rows_half // subs  # groups per sub-chunk
                for s in range(subs):
                    gs = (
                        g[:][:, 8 * sg * s : 8 * sg * (s + 1)]
                        .rearrange("p (h k) -> p h k", k=8)
                    )
                    u = tp.tile([P, sg, 4], f32, tag="u", name=f"u{row0}_{s}")
                    nc.vector.tensor_add(u[:], gs[:, :, 0::2], gs[:, :, 1::2])
                    w = tp.tile([P, sg, 2], f32, tag="w", name=f"w{row0}_{s}")
                    nc.vector.tensor_add(w[:], u[:][:, :, 0::2], u[:][:, :, 1::2])
                    m = tp.tile([P, sg], f32, tag="m", name=f"m{row0}_{s}")
                    mn = tp.tile([P, sg], f32, tag="n", name=f"n{row0}_{s}")
                    nc.vector.tensor_tensor(
                        m[:], w[:][:, :, 0], w[:][:, :, 1], op=Alu.max
                    )
                    nc.vector.tensor_tensor(
                        mn[:], w[:][:, :, 0], w[:][:, :, 1], op=Alu.min
                    )
                    nc.vector.tensor_tensor(mn[:], mn[:], m[:], op=Alu.subtract)
                    nc.scalar.activation(mn[:], mn[:], Act.Exp)
                    nc.scalar.activation(mn[:], mn[:], Act.Ln, bias=1.0)
                    h = tp.tile([P, sg], f32, tag="h", name=f"h{row0}_{s}")
                    nc.vector.tensor_add(h[:], m[:], mn[:])
                    ra = slice(row0 + sg * s, row0 + sg * (s + 1))
                    rb = slice(row0 + rows_half + sg * s, row0 + rows_half + sg * (s + 1))
                    if replicate:
                        nc.sync.dma_start(dst[:][0:B, ra], h[:][0:B, :])
                        nc.sync.dma_start(dst[:][B:P, rb], h[:][B:P, :])
                        nc.sync.dma_start(dst[:][B:P, ra], h[:][0:B, :])
                        nc.sync.dma_start(dst[:][0:B, rb], h[:][B:P, :])
                    else:  # dst is the DRAM output [64, n]
                        nc.sync.dma_start(dst[:][:, ra], h[:][0:B, :])
                        nc.sync.dma_start(dst[:][:, rb], h[:][B:P, :])

            # --- phase 1: fused layers 0+1 -> t2 ---
            hg = P1_GROUPS // 2  # groups per half per chunk (1024)
            for ci in range(P1_CHUNKS):
                g = gp.tile([P, SLOTS], f32, tag="g", name=f"g1_{ci}")
                nc.gpsimd.ap_gather(
                    g[:],
                    t0[:].rearrange("p (n d) -> p n d", d=1),
                    ix1[ci][:],
                    P,
                    TAB0,
                    1,
                    SLOTS,
                )
                row0 = P1_GROUPS * ci
                reduce8_lse(g, t2, row0, hg, replicate=True)

            # --- phase 2: fused layers 2+3 -> out ---
            g = gp.tile([P, SLOTS], f32, tag="g", name="g2")
            nc.gpsimd.ap_gather(
                g[:],
                t2[:].rearrange("p (n d) -> p n d", d=1),
                ix2[:],
                P,
                N_MID,
                1,
                SLOTS,
            )
            reduce8_lse(g, out_d, 0, N_OUT // 2, replicate=False)
    nc.compile()
    return nc


def host_prep(x, ptrs_list, seg_list, n_vars=N_VARS):
    """Host-side sharding + pointer-chain composition. Returns per-core
    input maps."""
    x = np.asarray(x, dtype=np.float32)
    p0, p1, p2, p3 = [np.asarray(p).astype(np.int64) for p in ptrs_list]
    for i, (n_out, f) in enumerate(zip(OUT_SIZES, FANINS)):
        seg = np.asarray(seg_list[i]).astype(np.int64)
        expected = np.repeat(np.arange(n_out, dtype=np.int64), f)
        assert np.array_equal(seg, expected), f"layer {i}: non-uniform segments"

    # remap ptr0 rows into the block layout [0, 0 | pos | neg]
    q0 = np.where(
        p0 < 2, p0, np.where(p0 % 2 == 0, 2 + (p0 - 2) // 2, 2 + n_vars + (p0 - 3) // 2)
    )

    k4 = np.arange(4)
    # phase 1: fused group g (t2 row g): 8 t0-rows q0[4*p1[2g]+k], q0[4*p1[2g+1]+k]
    a, b = p1[0::2], p1[1::2]
    g1 = np.concatenate(
        [q0[4 * a[:, None] + k4], q0[4 * b[:, None] + k4]], axis=1
    )  # [8192, 8]
    # phase 2: fused group h (out row h): 8 t2-rows p2[4*p3[2h]+k], p2[4*p3[2h+1]+k]
    c, d = p3[0::2], p3[1::2]
    g2 = np.concatenate(
        [p2[4 * c[:, None] + k4], p2[4 * d[:, None] + k4]], axis=1
    )  # [2048, 8]

    idx_maps = {}
    for ci in range(P1_CHUNKS):
        gr = g1[P1_GROUPS * ci : P1_GROUPS * (ci + 1)]
        hg = P1_GROUPS // 2
        idx_maps[f"idx1_{ci}"] = wrap128(gr[:hg].reshape(-1), gr[hg:].reshape(-1))
    idx_maps["idx2"] = wrap128(
        g2[: N_OUT // 2].reshape(-1), g2[N_OUT // 2 :].reshape(-1)
    )

    batch = x.shape[1]
    bpc = batch // NCORES
    in_maps = []
    for i in range(NCORES):
        xt = np.ascontiguousarray(x[:, i * bpc : (i + 1) * bpc].T)  # [64, 4096]
        xv = np.concatenate([xt, xt], axis=0)  # [128, 4096], both copies
        in_maps.append({"xv": xv, **idx_maps})
    return in_maps


_CACHE = {}


def _get_nc():
    if "nc" not in _CACHE:
        _CACHE["nc"] = build_nc()
    return _CACHE["nc"]


def kernel(x, ptrs0, seg0, ptrs1, seg1, ptrs2, seg2, ptrs3, seg3):
    from concourse.bass_utils import run_bass_kernel_spmd

    nc = _get_nc()
    in_maps = host_prep(
        x, [ptrs0, ptrs1, ptrs2, ptrs3], [seg0, seg1, seg2, seg3]
    )
    res = run_bass_kernel_spmd(nc, in_maps, core_ids=list(range(NCORES)))
    outs = [r["out"] for r in res.results]  # each [64, 2048]
    return np.concatenate([o.T for o in outs], axis=1).astype(np.float32)


# revision 16
# speedup vs baseline: 2.2809x; 1.0272x over previous
"""Trainium2 Bass kernel for the sum-product "knowledge layer" network.

Computation (see problem reference):
  h0 = encode(x): 8194-row table [-inf, 0, pos0, neg0, ...] with pos = x
       (log-probs), neg = log(1 - exp(x)), per batch column.
  4 alternating layers: gather rows by ptrs, segment-reduce over contiguous
  fanin groups (fanin-4 sum "product" layers, fanin-2 logsumexp "sum" layers).

Strategy (pure batch data-parallelism, 8 NeuronCores, SBUF-resident):
  - Shard the 512 batch columns 8 ways -> 64 columns per core.
  - Batch-on-partition layout: every table lives in SBUF as [128, n_rows]
    f32 where channel c holds batch column c%64 (two identical copies).
  - Gathers use the gpsimd ap_gather instruction (SBUF->SBUF, per-16-
    partition-group index lists): channels 0-63 gather edge-set A while
    64-127 gather edge-set B, so each free slot delivers 2 edges.
  - Layer fusion: t1 rows are consumed on average once by layer 1 and t3
    rows once by layer 3, so layers (0,1) and (2,3) are fused.  The host
    composes the pointer chains: one device gather phase fetches, per fused
    output group g, the 8 t0-rows {ptr0[4*ptr1[2g+j]+k]}, the device then
    computes lse(sum4, sum4).  Phase 2 repeats the pattern from t2.
  - Encode avoids interleaving: t0 = [2 const rows | pos block | neg block]
    and the host remaps ptr0 accordingly; x DMA-loads straight into the pos
    block, Act writes exp/log1mexp into the neg block.
"""

import numpy as np

P = 128
B = 64  # batch columns per core
NCORES = 8
N_VARS = 4096
BATCH = 512
TAB0 = 2 * N_VARS + 2  # 8194
OUT_SIZES = [16384, 8192, 4096, 2048]
FANINS = [4, 2, 4, 2]
N_MID = 8192  # t2 rows (output of fused phase 1)
N_OUT = 2048
P1_CHUNKS = 4
P1_GROUPS = N_MID // P1_CHUNKS  # fused groups per phase-1 chunk (2048)
SLOTS = 8192  # gather slots per ap_gather instruction


def wrap128(list_a, list_b):
    """Pack two per-half edge lists (len S each) into the ap_gather int16
    index layout [128, S//16]: position j of a group's list sits at
    [16*grp + j%16, j//16]; groups 0-3 share list A, 4-7 share list B."""
    a = np.asarray(list_a, np.int64)
    b = np.asarray(list_b, np.int64)
    assert a.size == b.size and a.size % 16 == 0
    wa = np.tile(a.reshape(-1, 16).T, (4, 1))
    wb = np.tile(b.reshape(-1, 16).T, (4, 1))
    w = np.concatenate([wa, wb], axis=0)
    assert w.min() >= 0 and w.max() < 2**15
    return np.ascontiguousarray(w.astype(np.int16))


def _patch_act_tables():
    """Make the combined exp+ln activation-function set the only candidate
    for Exp and Ln so the compiler emits a single LoadActFuncSet instead of
    ping-ponging between the exp-only and ln-only sets (1283ns per reload).
    Set ids (dict order) are preserved; the combined set genuinely contains
    both functions, so device behavior is unchanged."""
    import concourse.bacc as bacc
    import concourse.hw_specs as hws
    import concourse.mybir as mybir

    Act = mybir.ActivationFunctionType
    orig = hws.get_activation_tables

    def patched(arch):
        tabs = dict(orig(arch))
        out = {}
        for name, s in tabs.items():
            s2 = set(s)
            if name != "natural_log_exp_and_others":
                s2.discard(Act.Exp)
                s2.discard(Act.Ln)
            out[name] = s2
        return out

    bacc.get_activation_tables = patched


def build_nc():
    import concourse.bacc as bacc
    import concourse.mybir as mybir
    import concourse.tile as tile

    _patch_act_tables()

    f32 = mybir.dt.float32
    i16 = mybir.dt.int16
    Alu = mybir.AluOpType
    Act = mybir.ActivationFunctionType

    nc = bacc.Bacc("TRN2", target_bir_lowering=False, debug=False)
    xv_in = nc.dram_tensor("xv", [P, N_VARS], f32, kind="ExternalInput")
    idx1_in = [
        nc.dram_tensor(f"idx1_{c}", [P, SLOTS // 16], i16, kind="ExternalInput")
        for c in range(P1_CHUNKS)
    ]
    idx2_in = nc.dram_tensor("idx2", [P, SLOTS // 16], i16, kind="ExternalInput")
    out_d = nc.dram_tensor("out", [B, N_OUT], f32, kind="ExternalOutput")

    with tile.TileContext(nc) as tc:
        with (
            tc.tile_pool(name="tabs", bufs=1) as tabs,
            tc.tile_pool(name="gp", bufs=2) as gp,
            tc.tile_pool(name="tp", bufs=2) as tp,
            tc.tile_pool(name="ix", bufs=1) as ixp,
        ):
            t0 = tabs.tile([P, TAB0], f32, tag="t0")
            t2 = tabs.tile([P, N_MID], f32, tag="t2")

            # --- encode: t0 = [0, 0 | pos_0..pos_4095 | neg_0..neg_4095] ---
            # x loads go first (the encode chain is the critical-path head);
            # halves let Exp start while the second half is still in flight.
            half = N_VARS // 2
            nc.vector.memset(t0[:][:, 0:2], 0.0)
            for h in range(2):
                nc.sync.dma_start(
                    t0[:][:, 2 + half * h : 2 + half * (h + 1)],
                    xv_in[:][:, half * h : half * (h + 1)],
                )
            ix1 = [
                ixp.tile([P, SLOTS // 16], i16, tag=f"i{c}", name=f"ix1_{c}")
                for c in range(P1_CHUNKS)
            ]
            ix2 = ixp.tile([P, SLOTS // 16], i16, tag="i4")
            for c in range(P1_CHUNKS):
                nc.sync.dma_start(ix1[c][:], idx1_in[c][:])
            nc.sync.dma_start(ix2[:], idx2_in[:])
            for h in range(2):
                et = tp.tile([P, half], f32, tag="et", name=f"et{h}")
                pos = t0[:][:, 2 + half * h : 2 + half * (h + 1)]
                neg = t0[:][:, 2 + N_VARS + half * h : 2 + N_VARS + half * (h + 1)]
                nc.scalar.activation(et[:], pos, Act.Exp)
                nc.scalar.activation(neg, et[:], Act.Ln, scale=-1.0, bias=1.0)

            def reduce8_lse(g, dst, row0, rows_half, replicate, subs=2, pool_u=()):
                """g [128, 8*rows_half]: per channel-half, rows_half fused
                groups of 8 slots [a0..a3, b0..b3]; writes
                lse(a0+..+a3, b0+..+b3) to dst rows [row0, row0+2*rows_half)
                (half A from channels 0-63, half B from 64-127).  dst may be
                an SBUF table tile or a DRAM [64, n] output.  With
                replicate=True both channel copies of dst get every row (DMA
                placement copies); otherwise each half lands only on its own
                channels."""
                sg = rows_half // subs  # groups per sub-chunk
                for s in range(subs):
                    gs = (
                        g[:][:, 8 * sg * s : 8 * sg * (s + 1)]
                        .rearrange("p (h k) -> p h k", k=8)
                    )
                    u = tp.tile([P, sg, 4], f32, tag="u", name=f"u{row0}_{s}")
                    # when the Pool engine has no gathers left (drain/phase 2)
                    # it takes a sub-chunk's first-stage add off the DVE chain
                    eng = nc.gpsimd if s in pool_u else nc.vector
                    eng.tensor_add(u[:], gs[:, :, 0::2], gs[:, :, 1::2])
                    w = tp.tile([P, sg, 2], f32, tag="w", name=f"w{row0}_{s}")
                    nc.vector.tensor_add(w[:], u[:][:, :, 0::2], u[:][:, :, 1::2])
                    m = tp.tile([P, sg], f32, tag="m", name=f"m{row0}_{s}")
                    mn = tp.tile([P, sg], f32, tag="n", name=f"n{row0}_{s}")
                    nc.vector.tensor_tensor(
                        m[:], w[:][:, :, 0], w[:][:, :, 1], op=Alu.max
                    )
                    nc.vector.tensor_tensor(
                        mn[:], w[:][:, :, 0], w[:][:, :, 1], op=Alu.min
                    )
                    nc.vector.tensor_tensor(mn[:], mn[:], m[:], op=Alu.subtract)
                    nc.scalar.activation(mn[:], mn[:], Act.Exp)
                    nc.scalar.activation(mn[:], mn[:], Act.Ln, bias=1.0)
                    h = tp.tile([P, sg], f32, tag="h", name=f"h{row0}_{s}")
                    nc.vector.tensor_add(h[:], m[:], mn[:])
                    ra = slice(row0 + sg * s, row0 + sg * (s + 1))
                    rb = slice(row0 + rows_half + sg * s, row0 + rows_half + sg * (s + 1))
                    if replicate:
                        nc.sync.dma_start(dst[:][0:B, ra], h[:][0:B, :])
                        nc.sync.dma_start(dst[:][B:P, rb], h[:][B:P, :])
                        nc.sync.dma_start(dst[:][B:P, ra], h[:][0:B, :])
                        nc.sync.dma_start(dst[:][0:B, rb], h[:][B:P, :])
                    else:  # dst is the DRAM output [64, n]
                        nc.sync.dma_start(dst[:][:, ra], h[:][0:B, :])
                        nc.sync.dma_start(dst[:][:, rb], h[:][B:P, :])

            # --- phase 1: fused layers 0+1 -> t2 ---
            hg = P1_GROUPS // 2  # groups per half per chunk (1024)
            for ci in range(P1_CHUNKS):
                g = gp.tile([P, SLOTS], f32, tag="g", name=f"g1_{ci}")
                nc.gpsimd.ap_gather(
                    g[:],
                    t0[:].rearrange("p (n d) -> p n d", d=1),
                    ix1[ci][:],
                    P,
                    TAB0,
                    1,
                    SLOTS,
                )
                row0 = P1_GROUPS * ci
                reduce8_lse(
                    g,
                    t2,
                    row0,
                    hg,
                    replicate=True,
                    pool_u=(1,) if ci == P1_CHUNKS - 1 else (),
                )

            # --- phase 2: fused layers 2+3 -> out ---
            g = gp.tile([P, SLOTS], f32, tag="g", name="g2")
            nc.gpsimd.ap_gather(
                g[:],
                t2[:].rearrange("p (n d) -> p n d", d=1),
                ix2[:],
                P,
                N_MID,
                1,
                SLOTS,
            )
            reduce8_lse(g, out_d, 0, N_OUT // 2, replicate=False, pool_u=(1,))
    nc.compile()
    return nc


def host_prep(x, ptrs_list, seg_list, n_vars=N_VARS):
    """Host-side sharding + pointer-chain composition. Returns per-core
    input maps."""
    x = np.asarray(x, dtype=np.float32)
    p0, p1, p2, p3 = [np.asarray(p).astype(np.int64) for p in ptrs_list]
    for i, (n_out, f) in enumerate(zip(OUT_SIZES, FANINS)):
        seg = np.asarray(seg_list[i]).astype(np.int64)
        expected = np.repeat(np.arange(n_out, dtype=np.int64), f)
        assert np.array_equal(seg, expected), f"layer {i}: non-uniform segments"

    # remap ptr0 rows into the block layout [0, 0 | pos | neg]
    q0 = np.where(
        p0 < 2, p0, np.where(p0 % 2 == 0, 2 + (p0 - 2) // 2, 2 + n_vars + (p0 - 3) // 2)
    )

    k4 = np.arange(4)
    # phase 1: fused group g (t2 row g): 8 t0-rows q0[4*p1[2g]+k], q0[4*p1[2g+1]+k]
    a, b = p1[0::2], p1[1::2]
    g1 = np.concatenate(
        [q0[4 * a[:, None] + k4], q0[4 * b[:, None] + k4]], axis=1
    )  # [8192, 8]
    # phase 2: fused group h (out row h): 8 t2-rows p2[4*p3[2h]+k], p2[4*p3[2h+1]+k]
    c, d = p3[0::2], p3[1::2]
    g2 = np.concatenate(
        [p2[4 * c[:, None] + k4], p2[4 * d[:, None] + k4]], axis=1
    )  # [2048, 8]

    idx_maps = {}
    for ci in range(P1_CHUNKS):
        gr = g1[P1_GROUPS * ci : P1_GROUPS * (ci + 1)]
        hg = P1_GROUPS // 2
        idx_maps[f"idx1_{ci}"] = wrap128(gr[:hg].reshape(-1), gr[hg:].reshape(-1))
    idx_maps["idx2"] = wrap128(
        g2[: N_OUT // 2].reshape(-1), g2[N_OUT // 2 :].reshape(-1)
    )

    batch = x.shape[1]
    bpc = batch // NCORES
    in_maps = []
    for i in range(NCORES):
        xt = np.ascontiguousarray(x[:, i * bpc : (i + 1) * bpc].T)  # [64, 4096]
        xv = np.concatenate([xt, xt], axis=0)  # [128, 4096], both copies
        in_maps.append({"xv": xv, **idx_maps})
    return in_maps


_CACHE = {}


def _get_nc():
    if "nc" not in _CACHE:
        _CACHE["nc"] = build_nc()
    return _CACHE["nc"]


def kernel(x, ptrs0, seg0, ptrs1, seg1, ptrs2, seg2, ptrs3, seg3):
    from concourse.bass_utils import run_bass_kernel_spmd

    nc = _get_nc()
    in_maps = host_prep(
        x, [ptrs0, ptrs1, ptrs2, ptrs3], [seg0, seg1, seg2, seg3]
    )
    res = run_bass_kernel_spmd(nc, in_maps, core_ids=list(range(NCORES)))
    outs = [r["out"] for r in res.results]  # each [64, 2048]
    return np.concatenate([o.T for o in outs], axis=1).astype(np.float32)


# revision 23
# speedup vs baseline: 2.3376x; 1.0249x over previous
"""Trainium2 Bass kernel for the sum-product "knowledge layer" network.

Computation (see problem reference):
  h0 = encode(x): 8194-row table [-inf, 0, pos0, neg0, ...] with pos = x
       (log-probs), neg = log(1 - exp(x)), per batch column.
  4 alternating layers: gather rows by ptrs, segment-reduce over contiguous
  fanin groups (fanin-4 sum "product" layers, fanin-2 logsumexp "sum" layers).

Strategy (pure batch data-parallelism, 8 NeuronCores, SBUF-resident):
  - Shard the 512 batch columns 8 ways -> 64 columns per core.
  - Batch-on-partition layout: every table lives in SBUF as [128, n_rows]
    f32 where channel c holds batch column c%64 (two identical copies).
  - Gathers use the gpsimd ap_gather instruction (SBUF->SBUF, per-16-
    partition-group index lists): channels 0-63 gather edge-set A while
    64-127 gather edge-set B, so each free slot delivers 2 edges.
  - Layer fusion: t1 rows are consumed on average once by layer 1 and t3
    rows once by layer 3, so layers (0,1) and (2,3) are fused.  The host
    composes the pointer chains: one device gather phase fetches, per fused
    output group g, the 8 t0-rows {ptr0[4*ptr1[2g+j]+k]}, the device then
    computes lse(sum4, sum4).  Phase 2 repeats the pattern from t2.
  - Encode avoids interleaving: t0 = [2 const rows | pos block | neg block]
    and the host remaps ptr0 accordingly; x DMA-loads straight into the pos
    block, Act writes exp/log1mexp into the neg block.
"""

import numpy as np

P = 128
B = 64  # batch columns per core
NCORES = 8
N_VARS = 4096
BATCH = 512
TAB0 = 2 * N_VARS + 2  # 8194
OUT_SIZES = [16384, 8192, 4096, 2048]
FANINS = [4, 2, 4, 2]
N_MID = 8192  # t2 rows (output of fused phase 1)
N_OUT = 2048
P1_CHUNKS = 4
P1_GROUPS = N_MID // P1_CHUNKS  # fused groups per phase-1 chunk (2048)
SLOTS = 8192  # gather slots per ap_gather instruction


def wrap128(list_a, list_b):
    """Pack two per-half edge lists (len S each) into the ap_gather int16
    index layout [128, S//16]: position j of a group's list sits at
    [16*grp + j%16, j//16]; groups 0-3 share list A, 4-7 share list B."""
    a = np.asarray(list_a, np.int64)
    b = np.asarray(list_b, np.int64)
    assert a.size == b.size and a.size % 16 == 0
    wa = np.tile(a.reshape(-1, 16).T, (4, 1))
    wb = np.tile(b.reshape(-1, 16).T, (4, 1))
    w = np.concatenate([wa, wb], axis=0)
    assert w.min() >= 0 and w.max() < 2**15
    return np.ascontiguousarray(w.astype(np.int16))


def _patch_act_tables():
    """Make the combined exp+ln activation-function set the only candidate
    for Exp and Ln so the compiler emits a single LoadActFuncSet instead of
    ping-ponging between the exp-only and ln-only sets (1283ns per reload).
    Set ids (dict order) are preserved; the combined set genuinely contains
    both functions, so device behavior is unchanged."""
    import concourse.bacc as bacc
    import concourse.hw_specs as hws
    import concourse.mybir as mybir

    Act = mybir.ActivationFunctionType
    orig = hws.get_activation_tables

    def patched(arch):
        tabs = dict(orig(arch))
        out = {}
        for name, s in tabs.items():
            s2 = set(s)
            if name != "natural_log_exp_and_others":
                s2.discard(Act.Exp)
                s2.discard(Act.Ln)
            out[name] = s2
        return out

    bacc.get_activation_tables = patched


def build_nc():
    import concourse.bacc as bacc
    import concourse.mybir as mybir
    import concourse.tile as tile

    _patch_act_tables()

    f32 = mybir.dt.float32
    i16 = mybir.dt.int16
    Alu = mybir.AluOpType
    Act = mybir.ActivationFunctionType

    nc = bacc.Bacc("TRN2", target_bir_lowering=False, debug=False)
    xv_in = nc.dram_tensor("xv", [P, N_VARS], f32, kind="ExternalInput")
    idx1_in = [
        nc.dram_tensor(f"idx1_{c}", [P, SLOTS // 16], i16, kind="ExternalInput")
        for c in range(P1_CHUNKS)
    ]
    idx2_in = nc.dram_tensor("idx2", [P, SLOTS // 16], i16, kind="ExternalInput")
    out_d = nc.dram_tensor("out", [B, N_OUT], f32, kind="ExternalOutput")

    with tile.TileContext(nc) as tc:
        with (
            tc.tile_pool(name="tabs", bufs=1) as tabs,
            tc.tile_pool(name="gp", bufs=2) as gp,
            tc.tile_pool(name="tp", bufs=2) as tp,
            tc.tile_pool(name="ix", bufs=1) as ixp,
        ):
            t0 = tabs.tile([P, TAB0], f32, tag="t0")
            t2 = tabs.tile([P, N_MID], f32, tag="t2")

            # --- encode: t0 = [0, 0 | pos_0..pos_4095 | neg_0..neg_4095] ---
            # x loads go first (the encode chain is the critical-path head);
            # halves let Exp start while the second half is still in flight.
            nq = 4
            q = N_VARS // nq
            nc.vector.memset(t0[:][:, 0:2], 0.0)
            for h in range(nq):
                nc.sync.dma_start(
                    t0[:][:, 2 + q * h : 2 + q * (h + 1)],
                    xv_in[:][:, q * h : q * (h + 1)],
                )
            ix1 = [
                ixp.tile([P, SLOTS // 16], i16, tag=f"i{c}", name=f"ix1_{c}")
                for c in range(P1_CHUNKS)
            ]
            ix2 = ixp.tile([P, SLOTS // 16], i16, tag="i4")
            for c in range(P1_CHUNKS):
                nc.sync.dma_start(ix1[c][:], idx1_in[c][:])
            nc.sync.dma_start(ix2[:], idx2_in[:])
            for h in range(nq):
                et = tp.tile([P, q], f32, tag="et", name=f"et{h}")
                pos = t0[:][:, 2 + q * h : 2 + q * (h + 1)]
                neg = t0[:][:, 2 + N_VARS + q * h : 2 + N_VARS + q * (h + 1)]
                nc.scalar.activation(et[:], pos, Act.Exp)
                nc.scalar.activation(neg, et[:], Act.Ln, scale=-1.0, bias=1.0)

            def reduce8_lse(g, dst, row0, rows_half, replicate, subs=2, pool_u=(), tail=False):
                """g [128, 8*rows_half]: per channel-half, rows_half fused
                groups of 8 slots [a0..a3, b0..b3]; writes
                lse(a0+..+a3, b0+..+b3) to dst rows [row0, row0+2*rows_half)
                (half A from channels 0-63, half B from 64-127).  dst may be
                an SBUF table tile or a DRAM [64, n] output.  With
                replicate=True both channel copies of dst get every row (DMA
                placement copies); otherwise each half lands only on its own
                channels."""
                sg = rows_half // subs  # groups per sub-chunk
                for s in range(subs):
                    gs = (
                        g[:][:, 8 * sg * s : 8 * sg * (s + 1)]
                        .rearrange("p (h k) -> p h k", k=8)
                    )
                    u = tp.tile([P, sg, 4], f32, tag="u", name=f"u{row0}_{s}")
                    # when the Pool engine has no gathers left (drain/phase 2)
                    # it takes a sub-chunk's first-stage add off the DVE chain
                    eng = nc.gpsimd if s in pool_u else nc.vector
                    eng.tensor_add(u[:], gs[:, :, 0::2], gs[:, :, 1::2])
                    w = tp.tile([P, sg, 2], f32, tag="w", name=f"w{row0}_{s}")
                    nc.vector.tensor_add(w[:], u[:][:, :, 0::2], u[:][:, :, 1::2])
                    m = tp.tile([P, sg], f32, tag="m", name=f"m{row0}_{s}")
                    mn = tp.tile([P, sg], f32, tag="n", name=f"n{row0}_{s}")
                    nc.vector.tensor_tensor(
                        m[:], w[:][:, :, 0], w[:][:, :, 1], op=Alu.max
                    )
                    nc.vector.tensor_tensor(
                        mn[:], w[:][:, :, 0], w[:][:, :, 1], op=Alu.min
                    )
                    nc.vector.tensor_tensor(mn[:], mn[:], m[:], op=Alu.subtract)
                    nc.scalar.activation(mn[:], mn[:], Act.Exp)
                    nc.scalar.activation(mn[:], mn[:], Act.Ln, bias=1.0)
                    ra = slice(row0 + sg * s, row0 + sg * (s + 1))
                    rb = slice(row0 + rows_half + sg * s, row0 + rows_half + sg * (s + 1))
                    if replicate and tail and s == subs - 1:
                        # last sub sits on the critical path into the next
                        # gather: write own-copy halves with DVE directly and
                        # cross-replicate with just 2 DMAs
                        nc.vector.tensor_add(dst[:][0:B, ra], m[:][0:B, :], mn[:][0:B, :])
                        nc.vector.tensor_add(dst[:][B:P, rb], m[:][B:P, :], mn[:][B:P, :])
                        nc.sync.dma_start(dst[:][B:P, ra], dst[:][0:B, ra])
                        nc.sync.dma_start(dst[:][0:B, rb], dst[:][B:P, rb])
                        continue
                    h = tp.tile([P, sg], f32, tag="h", name=f"h{row0}_{s}")
                    nc.vector.tensor_add(h[:], m[:], mn[:])
                    if replicate:
                        nc.sync.dma_start(dst[:][0:B, ra], h[:][0:B, :])
                        nc.sync.dma_start(dst[:][B:P, rb], h[:][B:P, :])
                        nc.sync.dma_start(dst[:][B:P, ra], h[:][0:B, :])
                        nc.sync.dma_start(dst[:][0:B, rb], h[:][B:P, :])
                    else:  # dst is the DRAM output [64, n]
                        nc.sync.dma_start(dst[:][:, ra], h[:][0:B, :])
                        nc.sync.dma_start(dst[:][:, rb], h[:][B:P, :])

            # --- phase 1: fused layers 0+1 -> t2 ---
            hg = P1_GROUPS // 2  # groups per half per chunk (1024)
            for ci in range(P1_CHUNKS):
                g = gp.tile([P, SLOTS], f32, tag="g", name=f"g1_{ci}")
                nc.gpsimd.ap_gather(
                    g[:],
                    t0[:].rearrange("p (n d) -> p n d", d=1),
                    ix1[ci][:],
                    P,
                    TAB0,
                    1,
                    SLOTS,
                )
                row0 = P1_GROUPS * ci
                reduce8_lse(
                    g,
                    t2,
                    row0,
                    hg,
                    replicate=True,
                    pool_u=(1,) if ci == P1_CHUNKS - 1 else (),
                    tail=ci == P1_CHUNKS - 1,
                )

            # --- phase 2: fused layers 2+3 -> out ---
            g = gp.tile([P, SLOTS], f32, tag="g", name="g2")
            nc.gpsimd.ap_gather(
                g[:],
                t2[:].rearrange("p (n d) -> p n d", d=1),
                ix2[:],
                P,
                N_MID,
                1,
                SLOTS,
            )
            reduce8_lse(g, out_d, 0, N_OUT // 2, replicate=False, pool_u=(1,))
    nc.compile()
    return nc


def host_prep(x, ptrs_list, seg_list, n_vars=N_VARS):
    """Host-side sharding + pointer-chain composition. Returns per-core
    input maps."""
    x = np.asarray(x, dtype=np.float32)
    p0, p1, p2, p3 = [np.asarray(p).astype(np.int64) for p in ptrs_list]
    for i, (n_out, f) in enumerate(zip(OUT_SIZES, FANINS)):
        seg = np.asarray(seg_list[i]).astype(np.int64)
        expected = np.repeat(np.arange(n_out, dtype=np.int64), f)
        assert np.array_equal(seg, expected), f"layer {i}: non-uniform segments"

    # remap ptr0 rows into the block layout [0, 0 | pos | neg]
    q0 = np.where(
        p0 < 2, p0, np.where(p0 % 2 == 0, 2 + (p0 - 2) // 2, 2 + n_vars + (p0 - 3) // 2)
    )

    k4 = np.arange(4)
    # phase 1: fused group g (t2 row g): 8 t0-rows q0[4*p1[2g]+k], q0[4*p1[2g+1]+k]
    a, b = p1[0::2], p1[1::2]
    g1 = np.concatenate(
        [q0[4 * a[:, None] + k4], q0[4 * b[:, None] + k4]], axis=1
    )  # [8192, 8]
    # phase 2: fused group h (out row h): 8 t2-rows p2[4*p3[2h]+k], p2[4*p3[2h+1]+k]
    c, d = p3[0::2], p3[1::2]
    g2 = np.concatenate(
        [p2[4 * c[:, None] + k4], p2[4 * d[:, None] + k4]], axis=1
    )  # [2048, 8]

    idx_maps = {}
    for ci in range(P1_CHUNKS):
        gr = g1[P1_GROUPS * ci : P1_GROUPS * (ci + 1)]
        hg = P1_GROUPS // 2
        idx_maps[f"idx1_{ci}"] = wrap128(gr[:hg].reshape(-1), gr[hg:].reshape(-1))
    idx_maps["idx2"] = wrap128(
        g2[: N_OUT // 2].reshape(-1), g2[N_OUT // 2 :].reshape(-1)
    )

    batch = x.shape[1]
    bpc = batch // NCORES
    in_maps = []
    for i in range(NCORES):
        xt = np.ascontiguousarray(x[:, i * bpc : (i + 1) * bpc].T)  # [64, 4096]
        xv = np.concatenate([xt, xt], axis=0)  # [128, 4096], both copies
        in_maps.append({"xv": xv, **idx_maps})
    return in_maps


_CACHE = {}


def _get_nc():
    if "nc" not in _CACHE:
        _CACHE["nc"] = build_nc()
    return _CACHE["nc"]


def kernel(x, ptrs0, seg0, ptrs1, seg1, ptrs2, seg2, ptrs3, seg3):
    from concourse.bass_utils import run_bass_kernel_spmd

    nc = _get_nc()
    in_maps = host_prep(
        x, [ptrs0, ptrs1, ptrs2, ptrs3], [seg0, seg1, seg2, seg3]
    )
    res = run_bass_kernel_spmd(nc, in_maps, core_ids=list(range(NCORES)))
    outs = [r["out"] for r in res.results]  # each [64, 2048]
    return np.concatenate([o.T for o in outs], axis=1).astype(np.float32)
